# revision 1
# baseline (speedup 1.0000x reference)
"""Trainium2 Bass kernel for nn_CombinedModel_wGCN (GNN message passing).

Reference computation per event b (B=4096 events, N=128 particles):
  x = concat(feat, emb_table[pdg])          [128, 16]
  x = x @ W_in + b_in                       [128, 128]
  6x: x = relu(x @ W_h[l] + b_h[l]); x = adj @ x
  out[b] = (mask-weighted mean_i x) @ W_out + b_out

Strategy (pure data-parallel over 8 cores, 512 events each, groups of 4):
  - State kept transposed per event: Xh_e = x_e^T [d, i] (bf16). The dense
    layer is per-event matmul(lhsT=Xh_e, rhs=W_h[l]) producing [j, d'] —
    which feeds the aggregation matmul(lhsT=R_e, rhs=adjT_e) directly, so
    the whole layer chain needs NO transposes.
  - Accuracy: bf16 activations, weights split into bf16 hi+lo pairs
    accumulated in PSUM (x@W_hi + x@W_lo), adjacency in bf16. Emulated
    end-to-end error ~4.5e-3 vs f32 reference.
  - Bias b_h: reference uses zeros; if nonzero at runtime, a rank-1
    matmul (ones ⊗ b) accumulates bias into the dense PSUM before relu.
  - Masked-mean pooling folds into v = adj^T (mask/denom) (host), so the
    last aggregation is an N=1 matmul per event into a persistent PSUM
    bank; the final W_out projection runs once in f32r over all 512
    pooled columns.
  - Groups are emitted in a 3-wide software-pipelined wavefront so the
    PE always has independent (LDWEIGHTS, MATMUL) pairs in flight.
"""

import os
import numpy as np
import ml_dtypes

B, N = 4096, 128
NUM_FEAT, EMBED = 8, 8
UNITS = 128
HIDDEN = 6
NCORES = 8
BC = B // NCORES  # events per core
G = 4  # events per group (one PSUM bank of 512 f32 columns)
NG = BC // G
D0 = NUM_FEAT + EMBED + 1  # input features augmented with ones row (b_in)
WF = 3  # wavefront width (groups in flight)

_cache = {}


def _build_nc(ngroups, has_bias):
    import concourse.tile as tile
    from concourse import mybir, bacc

    f32 = mybir.dt.float32
    f32r = mybir.dt.float32r
    bf16 = mybir.dt.bfloat16
    Relu = mybir.ActivationFunctionType.Relu
    GW = G * 128

    nc = bacc.Bacc(
        trn_type="TRN2", target_bir_lowering=False, debug=False, num_devices=NCORES
    )
    d_adjt = nc.declare_dram_parameter("adjt", [NG, 128, GW], bf16, isOutput=False)
    d_x0t = nc.declare_dram_parameter("x0t", [NG, D0, GW], bf16, isOutput=False)
    d_vt = nc.declare_dram_parameter("vt", [128, BC], bf16, isOutput=False)
    d_whh = nc.declare_dram_parameter("whh", [HIDDEN, 128, 128], bf16, isOutput=False)
    d_whl = nc.declare_dram_parameter("whl", [HIDDEN, 128, 128], bf16, isOutput=False)
    d_winh = nc.declare_dram_parameter("winh", [D0, 128], bf16, isOutput=False)
    d_winl = nc.declare_dram_parameter("winl", [D0, 128], bf16, isOutput=False)
    d_bh = nc.declare_dram_parameter("bh", [HIDDEN, 128], f32, isOutput=False)
    d_wout = nc.declare_dram_parameter("wout", [2, 128, 1], bf16, isOutput=False)
    d_bout = nc.declare_dram_parameter("bout", [1, 1], f32, isOutput=False)
    d_out = nc.declare_dram_parameter("out", [1, BC], f32, isOutput=True)

    with tile.TileContext(nc) as tc:
        with (
            tc.tile_pool(name="const", bufs=1) as constp,
            tc.tile_pool(name="adj", bufs=2 * WF + 2) as adjp,
            tc.tile_pool(name="x0", bufs=WF + 2) as x0p,
            tc.tile_pool(name="work", bufs=2 * WF) as workp,
            tc.tile_pool(name="psd", bufs=WF, space="PSUM") as psd,
            tc.tile_pool(name="psa", bufs=WF, space="PSUM") as psa,
            tc.tile_pool(name="psfix", bufs=1, space="PSUM") as psfixp,
        ):
            # ---- constants ----
            whh, whl = [], []
            for l in range(HIDDEN):
                th = constp.tile([128, 128], bf16, tag=f"whh{l}")
                nc.sync.dma_start(th[:], d_whh[l])
                whh.append(th)
                tl = constp.tile([128, 128], bf16, tag=f"whl{l}")
                nc.sync.dma_start(tl[:], d_whl[l])
                whl.append(tl)
            winh = constp.tile([D0, 128], bf16, tag="winh")
            nc.sync.dma_start(winh[:], d_winh[:])
            winl = constp.tile([D0, 128], bf16, tag="winl")
            nc.sync.dma_start(winl[:], d_winl[:])
            wouth = constp.tile([128, 1], bf16, tag="wouth")
            nc.sync.dma_start(wouth[:], d_wout[0])
            woutl = constp.tile([128, 1], bf16, tag="woutl")
            nc.sync.dma_start(woutl[:], d_wout[1])
            boutt = constp.tile([1, 1], f32, tag="bout")
            nc.sync.dma_start(boutt[:], d_bout[:])
            vsb = constp.tile([128, BC], bf16, tag="vsb")
            nc.sync.dma_start(vsb[:], d_vt[:])
            brow = []
            if has_bias:
                ones_row = constp.tile([1, 128], bf16, tag="ones_row")
                nc.vector.memset(ones_row[:], 1.0)
                for l in range(HIDDEN):
                    bst = constp.tile([1, 128], f32, tag=f"bst{l}")
                    nc.sync.dma_start(bst[:], d_bh[l].rearrange("(o d) -> o d", o=1))
                    bb = constp.tile([1, GW], bf16, tag=f"brow{l}")
                    for e in range(G):
                        nc.vector.tensor_copy(bb[:, e * 128 : (e + 1) * 128], bst[:])
                    brow.append(bb)

            pooled = psfixp.tile([128, BC], f32, tag="pooled")

            def copy_into(i, dst, src):
                if i % 2 == 0:
                    nc.scalar.copy(dst, src)
                else:
                    nc.vector.tensor_copy(dst, src)

            def relu_into(i, dst, src):
                if i % 2 == 0:
                    nc.scalar.activation(dst, src, Relu)
                else:
                    nc.vector.tensor_scalar_max(dst, src, 0.0)

            # ---- wavefront over groups of G events ----
            xh = {}
            for gb in range(0, ngroups, WF):
                gs = range(gb, min(gb + WF, ngroups))
                for g in gs:
                    adjt = adjp.tile([128, GW], bf16, tag="adjt")
                    nc.sync.dma_start(adjt[:], d_adjt[g])
                    x0t = x0p.tile([D0, GW], bf16, tag="x0t")
                    nc.sync.dma_start(x0t[:], d_x0t[g])
                    pin = psd.tile([128, GW], f32, tag="dense")
                    nc.tensor.matmul(pin[:], winh[:], x0t[:], start=True, stop=False)
                    nc.tensor.matmul(pin[:], winl[:], x0t[:], start=False, stop=True)
                    t = workp.tile([128, GW], bf16, tag="xh")
                    copy_into(g, t[:], pin[:])
                    xh[g] = (t, adjt)

                for l in range(HIDDEN):
                    pd = {}
                    for g in gs:
                        t, adjt = xh[g]
                        p = psd.tile([128, GW], f32, tag="dense")
                        for e in range(G):
                            s = slice(e * 128, (e + 1) * 128)
                            nc.tensor.matmul(
                                p[:, s], t[:, s], whh[l][:], start=True, stop=False
                            )
                            nc.tensor.matmul(
                                p[:, s], t[:, s], whl[l][:],
                                start=False, stop=not has_bias,
                            )
                        if has_bias:
                            nc.tensor.matmul(
                                p[:], ones_row[:], brow[l][:], start=False, stop=True,
                                skip_group_check=True,
                            )
                        pd[g] = p
                    rr = {}
                    for g in gs:
                        r = workp.tile([128, GW], bf16, tag="r")
                        relu_into(g + l, r[:], pd[g][:])
                        rr[g] = r
                    if l < HIDDEN - 1:
                        pa = {}
                        for g in gs:
                            _, adjt = xh[g]
                            p = psa.tile([128, GW], f32, tag="agg")
                            for e in range(G):
                                s = slice(e * 128, (e + 1) * 128)
                                nc.tensor.matmul(
                                    p[:, s], rr[g][:, s], adjt[:, s],
                                    start=True, stop=True,
                                )
                            pa[g] = p
                        for g in gs:
                            t = workp.tile([128, GW], bf16, tag="xh")
                            copy_into(g + l + 1, t[:], pa[g][:])
                            xh[g] = (t, xh[g][1])
                    else:
                        for g in gs:
                            for e in range(G):
                                s = slice(e * 128, (e + 1) * 128)
                                ev = g * G + e
                                nc.tensor.matmul(
                                    pooled[:, ev : ev + 1],
                                    rr[g][:, s],
                                    vsb[:, ev : ev + 1],
                                    start=True, stop=True,
                                )
                xh.clear()

            # ---- final projection: out = pooled^T @ W_out + b_out ----
            # bf16 hi/lo split keeps the whole kernel fp32-free (FWL-friendly)
            psb = constp.tile([128, BC], f32, tag="psb")
            nc.vector.tensor_copy(psb[:], pooled[:])
            phi = constp.tile([128, BC], bf16, tag="phi")
            nc.scalar.copy(phi[:], psb[:])
            plo = constp.tile([128, BC], bf16, tag="plo")
            nc.vector.tensor_tensor(
                plo[:], psb[:], phi[:], mybir.AluOpType.subtract
            )
            pout = psfixp.tile([1, BC], f32, tag="pooled")
            nc.tensor.matmul(pout[:], wouth[:], phi[:], start=True, stop=False)
            nc.tensor.matmul(pout[:], wouth[:], plo[:], start=False, stop=False)
            nc.tensor.matmul(pout[:], woutl[:], phi[:], start=False, stop=True)
            outsb = constp.tile([1, BC], f32, tag="outsb")
            nc.vector.tensor_scalar_add(outsb[:], pout[:], boutt[:])
            nc.sync.dma_start(d_out[:], outsb[:])

    nc.finalize()
    return nc


def _split2(w):
    bf = ml_dtypes.bfloat16
    hi = w.astype(bf)
    lo = (w - hi.astype(np.float32)).astype(bf)
    return hi, lo


def _prep_inputs(pdg, feat, adj, mask, emb_table, W_in, b_in, W_h, b_h, W_out, b_out):
    bf = ml_dtypes.bfloat16
    pdg = np.asarray(pdg)
    feat = np.asarray(feat, dtype=np.float32)
    adj = np.asarray(adj, dtype=np.float32)
    mask = np.asarray(mask, dtype=np.float32)
    emb_table = np.asarray(emb_table, dtype=np.float32)

    emb = emb_table[pdg]  # [B, N, EMBED]
    ones = np.ones((B, N, 1), dtype=np.float32)
    x0 = np.concatenate([feat, emb, ones], axis=-1)  # [B, N, 17]
    x0t = x0.transpose(0, 2, 1)  # [B, 17, N]
    x0t4 = (
        np.ascontiguousarray(x0t.reshape(B // G, G, D0, N).transpose(0, 2, 1, 3))
        .reshape(B // G, D0, G * N)
        .astype(bf)
    )

    adjt = adj.transpose(0, 2, 1).astype(bf)  # [B, j, i]
    adjt4 = np.ascontiguousarray(
        adjt.reshape(B // G, G, N, N).transpose(0, 2, 1, 3)
    ).reshape(B // G, N, G * N)

    denom = np.clip(mask.sum(axis=1, keepdims=True), 1.0, None)
    m_scaled = (mask / denom).astype(np.float32)  # [B, N]
    v = np.matmul(m_scaled[:, None, :], adj).squeeze(1)  # [B, N]
    vt = v.T.astype(bf)  # [N, B]

    win_aug = np.concatenate(
        [np.asarray(W_in, np.float32), np.asarray(b_in, np.float32)[None, :]], axis=0
    )  # [17, 128]
    winh, winl = _split2(win_aug)
    whh, whl = _split2(np.asarray(W_h, np.float32))
    wouth, woutl = _split2(np.asarray(W_out, np.float32).reshape(128, 1))
    wout2 = np.stack([wouth, woutl])  # [2, 128, 1] bf16

    in_maps = []
    for c in range(NCORES):
        ev = slice(c * BC, (c + 1) * BC)
        gv = slice(c * (BC // G), (c + 1) * (BC // G))
        in_maps.append(
            {
                "adjt": adjt4[gv],
                "x0t": x0t4[gv],
                "vt": np.ascontiguousarray(vt[:, ev]),
                "whh": whh,
                "whl": whl,
                "winh": winh,
                "winl": winl,
                "bh": np.asarray(b_h, np.float32),
                "wout": wout2,
                "bout": np.asarray(b_out, np.float32).reshape(1, 1),
            }
        )
    return in_maps


def kernel(pdg, feat, adj, mask, emb_table, W_in, b_in, W_h, b_h, W_out, b_out):
    from concourse.bass_utils import run_bass_kernel_spmd

    ngroups = int(os.environ.get("KERNEL_NGROUPS", NG))
    has_bias = bool(np.any(np.asarray(b_h)))
    key = ("nc", ngroups, has_bias)
    if key not in _cache:
        _cache[key] = _build_nc(ngroups, has_bias)
    nc = _cache[key]

    in_maps = _prep_inputs(
        pdg, feat, adj, mask, emb_table, W_in, b_in, W_h, b_h, W_out, b_out
    )
    trace = bool(int(os.environ.get("KERNEL_TRACE", "0")))
    if trace:
        try:
            tmpdir = os.environ.get("KERNEL_TRACE_DIR") or None
            res = run_bass_kernel_spmd(
                nc, in_maps, core_ids=list(range(NCORES)), trace=True, tmpdir=tmpdir
            )
            _cache["last_exec_time_ns"] = res.exec_time_ns
            _cache["last_results"] = res
        except Exception as e:
            print(f"trace run failed ({type(e).__name__}: {e}); rerunning untraced")
            _cache["last_exec_time_ns"] = None
            res = run_bass_kernel_spmd(nc, in_maps, core_ids=list(range(NCORES)))
    else:
        res = run_bass_kernel_spmd(nc, in_maps, core_ids=list(range(NCORES)))
    out = np.concatenate([res.results[c]["out"].reshape(BC) for c in range(NCORES)])
    return out.reshape(B, 1).astype(np.float32)



# revision 5
# speedup vs baseline: 1.4743x; 1.4743x over previous
"""Trainium2 Bass kernel for nn_CombinedModel_wGCN (GNN message passing).

Reference computation per event b (B=4096 events, N=128 particles):
  x = concat(feat, emb_table[pdg])          [128, 16]
  x = x @ W_in + b_in                       [128, 128]
  6x: x = relu(x @ W_h[l] + b_h[l]); x = adj @ x
  out[b] = (mask-weighted mean_i x) @ W_out + b_out

Strategy (pure data-parallel over 8 cores, 512 events each, groups of 8):
  - State kept transposed per event: Xh_e = x_e^T [d, i] (fp16). The dense
    layer is per-event matmul(lhsT=Xh_e, rhs=W_h[l]) producing [i, d'] -
    which feeds the aggregation matmul(lhsT=R_e, rhs=adjT_e) directly, so
    the whole layer chain needs NO transposes.
  - Precision: everything fp16. W_h[l] is pre-scaled by S=2^-5 (exact power
    of two - mantissa and hence quantization error untouched) to keep
    activations O(1); relu is positively homogeneous and b_h scales along,
    and the cumulative S^6 is divided back out of W_out on the host. fp16
    weights carry 11 mantissa bits, enough that a SINGLE dense matmul
    replaces the bf16 hi+lo pair (emulated end-to-end error ~3.7e-3 vs the
    f32 reference, vs 4.5e-3 for the bf16 hi/lo baseline).
  - The input linear layer has no relu before the first hidden dense, so
    W_in folds into layer 0 on the host: Wc = [W_in; b_in] @ W_h[0] * S
    (f64 product, then fp16). Layer 0 consumes the DMA'd x0^T [17, i]
    directly as a K=17 matmul - the input-layer matmuls AND their PSUM
    evacuation passes disappear entirely.
  - Bias b_h: reference uses zeros; if nonzero at runtime, a rank-1
    matmul (ones (x) b*S^(l+1)) accumulates bias into the dense PSUM.
  - Masked-mean pooling folds into v = adj^T (mask/denom) (host), so the
    last aggregation is an N=1 matmul per event into a persistent PSUM
    bank; the final W_out projection runs once over all 512 pooled columns.
  - PSUM->SBUF evacuations (relu of the dense output, cast-copy of the
    aggregation output) bound the kernel together with the PE. Only the
    Activation and DVE engines can read PSUM (the BIR verifier rejects
    GpSimd ops on PSUM), so passes alternate between them, and groups are
    sized G=8 ([128,1024] passes spanning two PSUM banks) to amortize each
    engine's fixed PSUM/SBUF access latency over more columns.
  - Groups are emitted in a 3-wide wavefront over a 3-buffer rotating PSUM
    pool (2 banks per tile) + 1 persistent pooled bank + 1 spare.
"""

import os
import numpy as np
import ml_dtypes

B, N = 4096, 128
NUM_FEAT, EMBED = 8, 8
UNITS = 128
HIDDEN = 6
VOCAB = 42
NCORES = 8
BC = B // NCORES  # events per core
G = 8  # events per group (one [128, 1024] f32 PSUM tile = 2 banks)
NG = BC // G
D0 = NUM_FEAT + EMBED + 1  # input features augmented with ones row (b_in)
WF = 3  # wavefront width (groups in flight)
SCALE = 2.0 ** -5  # per-layer weight scale keeping fp16 activations O(1)

_cache = {}


def _build_nc(ngroups, has_bias):
    import concourse.tile as tile
    from concourse import mybir, bacc

    f32 = mybir.dt.float32
    f16 = mybir.dt.float16
    bf16 = mybir.dt.bfloat16
    Relu = mybir.ActivationFunctionType.Relu
    GW = G * 128

    nc = bacc.Bacc(
        trn_type="TRN2", target_bir_lowering=False, debug=False, num_devices=NCORES
    )
    d_adjt = nc.declare_dram_parameter("adjt", [NG, 128, GW], f16, isOutput=False)
    d_x0t = nc.declare_dram_parameter("x0t", [NG, D0, GW], f16, isOutput=False)
    d_vt = nc.declare_dram_parameter("vt", [128, BC], f16, isOutput=False)
    d_wh = nc.declare_dram_parameter("wh", [HIDDEN, 128, 128], f16, isOutput=False)
    d_bh = nc.declare_dram_parameter("bh", [HIDDEN, 128], f32, isOutput=False)
    d_wout = nc.declare_dram_parameter("wout", [2, 128, 1], bf16, isOutput=False)
    d_bout = nc.declare_dram_parameter("bout", [1, 1], f32, isOutput=False)
    d_out = nc.declare_dram_parameter("out", [1, BC], f32, isOutput=True)

    with tile.TileContext(nc) as tc:
        with (
            tc.tile_pool(name="const", bufs=1) as constp,
            tc.tile_pool(name="adj", bufs=WF + 2) as adjp,
            tc.tile_pool(name="x0", bufs=WF + 2) as x0p,
            tc.tile_pool(name="work", bufs=2 * WF + 2) as workp,
            tc.tile_pool(name="ps", bufs=WF, space="PSUM") as psp,
            tc.tile_pool(name="psfix", bufs=1, space="PSUM") as psfixp,
        ):
            # ---- constants ----
            # wh[0] is the host-fused [17, 128] Wc; wh[1..5] are [128, 128]
            whc = constp.tile([D0, 128], f16, tag="whc")
            nc.sync.dma_start(whc[:], d_wh[0][:D0, :])
            wh = [whc]
            for l in range(1, HIDDEN):
                t = constp.tile([128, 128], f16, tag=f"wh{l}")
                nc.sync.dma_start(t[:], d_wh[l])
                wh.append(t)
            wouth = constp.tile([128, 1], bf16, tag="wouth")
            nc.sync.dma_start(wouth[:], d_wout[0])
            woutl = constp.tile([128, 1], bf16, tag="woutl")
            nc.sync.dma_start(woutl[:], d_wout[1])
            boutt = constp.tile([1, 1], f32, tag="bout")
            nc.sync.dma_start(boutt[:], d_bout[:])
            vsb = constp.tile([128, BC], f16, tag="vsb")
            nc.sync.dma_start(vsb[:], d_vt[:])
            brow = []
            if has_bias:
                ones_row = constp.tile([1, 128], f16, tag="ones_row")
                nc.vector.memset(ones_row[:], 1.0)
                for l in range(HIDDEN):
                    bst = constp.tile([1, 128], f32, tag=f"bst{l}")
                    nc.sync.dma_start(bst[:], d_bh[l].rearrange("(o d) -> o d", o=1))
                    bb = constp.tile([1, GW], f16, tag=f"brow{l}")
                    for e in range(G):
                        nc.vector.tensor_copy(bb[:, e * 128 : (e + 1) * 128], bst[:])
                    brow.append(bb)

            pooled = psfixp.tile([128, BC], f32, tag="pooled")

            # Evacuations alternate between the only two PSUM-capable
            # engines (Activation and DVE).
            evac_ctr = [0]

            def evac(dst, src, relu):
                eng = evac_ctr[0] % 2
                evac_ctr[0] += 1
                if relu:
                    if eng == 0:
                        nc.scalar.activation(dst, src, Relu)
                    else:
                        nc.vector.tensor_scalar_max(dst, src, 0.0)
                else:
                    if eng == 0:
                        nc.scalar.copy(dst, src)
                    else:
                        nc.vector.tensor_copy(dst, src)

            # ---- wavefront over groups of G events ----
            xh = {}
            adjt = {}
            for gb in range(0, ngroups, WF):
                gs = range(gb, min(gb + WF, ngroups))
                for g in gs:
                    at = adjp.tile([128, GW], f16, tag="adjt")
                    nc.sync.dma_start(at[:], d_adjt[g])
                    adjt[g] = at
                    x0t = x0p.tile([D0, GW], f16, tag="x0t")
                    nc.sync.dma_start(x0t[:], d_x0t[g])
                    xh[g] = x0t  # layer 0 consumes x0^T directly (fused W_in)

                for l in range(HIDDEN):
                    pd = {}
                    for g in gs:
                        p = psp.tile([128, GW], f32, tag="ps")
                        for e in range(G):
                            s = slice(e * 128, (e + 1) * 128)
                            nc.tensor.matmul(
                                p[:, s], xh[g][:, s], wh[l][:],
                                start=True, stop=not has_bias,
                            )
                        if has_bias:
                            nc.tensor.matmul(
                                p[:], ones_row[:], brow[l][:], start=False, stop=True,
                                skip_group_check=True,
                            )
                        pd[g] = p
                    rr = {}
                    for g in gs:
                        r = workp.tile([128, GW], f16, tag="r")
                        evac(r[:], pd[g][:], relu=True)
                        rr[g] = r
                    if l < HIDDEN - 1:
                        pa = {}
                        for g in gs:
                            p = psp.tile([128, GW], f32, tag="ps")
                            for e in range(G):
                                s = slice(e * 128, (e + 1) * 128)
                                nc.tensor.matmul(
                                    p[:, s], rr[g][:, s], adjt[g][:, s],
                                    start=True, stop=True,
                                )
                            pa[g] = p
                        for g in gs:
                            t = workp.tile([128, GW], f16, tag="xh")
                            evac(t[:], pa[g][:], relu=False)
                            xh[g] = t
                    else:
                        for g in gs:
                            for e in range(G):
                                s = slice(e * 128, (e + 1) * 128)
                                ev = g * G + e
                                nc.tensor.matmul(
                                    pooled[:, ev : ev + 1],
                                    rr[g][:, s],
                                    vsb[:, ev : ev + 1],
                                    start=True, stop=True,
                                )
                xh.clear()
                adjt.clear()

            # ---- final projection: out = pooled^T @ W_out + b_out ----
            psb = constp.tile([128, BC], f32, tag="psb")
            nc.vector.tensor_copy(psb[:], pooled[:])
            phi = constp.tile([128, BC], bf16, tag="phi")
            nc.scalar.copy(phi[:], psb[:])
            plo = constp.tile([128, BC], bf16, tag="plo")
            nc.vector.tensor_tensor(
                plo[:], psb[:], phi[:], mybir.AluOpType.subtract
            )
            pout = psfixp.tile([1, BC], f32, tag="pooled")
            nc.tensor.matmul(pout[:], wouth[:], phi[:], start=True, stop=False)
            nc.tensor.matmul(pout[:], wouth[:], plo[:], start=False, stop=False)
            nc.tensor.matmul(pout[:], woutl[:], phi[:], start=False, stop=True)
            outsb = constp.tile([1, BC], f32, tag="outsb")
            nc.vector.tensor_scalar_add(outsb[:], pout[:], boutt[:])
            nc.sync.dma_start(d_out[:], outsb[:])

    nc.finalize()
    return nc


def _split2(w, dt):
    hi = w.astype(dt)
    lo = (w - hi.astype(np.float32)).astype(dt)
    return hi, lo


def _prep_inputs(pdg, feat, adj, mask, emb_table, W_in, b_in, W_h, b_h, W_out, b_out):
    bf = ml_dtypes.bfloat16
    f16 = np.float16
    pdg = np.asarray(pdg)
    feat = np.asarray(feat, dtype=np.float32)
    adj = np.asarray(adj, dtype=np.float32)
    mask = np.asarray(mask, dtype=np.float32)
    emb_table = np.asarray(emb_table, dtype=np.float32)
    W_h = np.asarray(W_h, np.float32)

    emb = emb_table[pdg]  # [B, N, EMBED]
    ones = np.ones((B, N, 1), dtype=np.float32)
    x0 = np.concatenate([feat, emb, ones], axis=-1)  # [B, N, 17]
    x0t = x0.transpose(0, 2, 1)  # [B, 17, N]
    x0t4 = (
        np.ascontiguousarray(x0t.reshape(B // G, G, D0, N).transpose(0, 2, 1, 3))
        .reshape(B // G, D0, G * N)
        .astype(f16)
    )

    adjt = adj.transpose(0, 2, 1).astype(f16)  # [B, j, i]
    adjt4 = np.ascontiguousarray(
        adjt.reshape(B // G, G, N, N).transpose(0, 2, 1, 3)
    ).reshape(B // G, N, G * N)

    denom = np.clip(mask.sum(axis=1, keepdims=True), 1.0, None)
    m_scaled = (mask / denom).astype(np.float32)  # [B, N]
    v = np.matmul(m_scaled[:, None, :], adj).squeeze(1)  # [B, N]
    vt = v.T.astype(f16)  # [N, B]

    # Fuse the input linear into layer 0: Wc = [W_in; b_in] @ W_h[0] * S
    win_aug = np.concatenate(
        [np.asarray(W_in, np.float64), np.asarray(b_in, np.float64)[None, :]], axis=0
    )  # [17, 128]
    Wc = (win_aug @ W_h[0].astype(np.float64) * SCALE).astype(np.float32)
    whs = np.zeros((HIDDEN, 128, 128), np.float32)
    whs[0, :D0, :] = Wc
    whs[1:] = W_h[1:] * SCALE
    whs = whs.astype(f16)
    # scale bias rows to match the cumulative activation scale S^(l+1)
    bhs = np.asarray(b_h, np.float32) * (
        SCALE ** np.arange(1, HIDDEN + 1, dtype=np.float32)[:, None]
    )
    wout_unscaled = np.asarray(W_out, np.float32).reshape(128, 1) / (SCALE ** HIDDEN)
    wouth, woutl = _split2(wout_unscaled, bf)
    wout2 = np.stack([wouth, woutl])  # [2, 128, 1] bf16

    in_maps = []
    for c in range(NCORES):
        ev = slice(c * BC, (c + 1) * BC)
        gv = slice(c * (BC // G), (c + 1) * (BC // G))
        in_maps.append(
            {
                "adjt": adjt4[gv],
                "x0t": x0t4[gv],
                "vt": np.ascontiguousarray(vt[:, ev]),
                "wh": whs,
                "bh": bhs,
                "wout": wout2,
                "bout": np.asarray(b_out, np.float32).reshape(1, 1),
            }
        )
    return in_maps


def kernel(pdg, feat, adj, mask, emb_table, W_in, b_in, W_h, b_h, W_out, b_out):
    from concourse.bass_utils import run_bass_kernel_spmd

    ngroups = int(os.environ.get("KERNEL_NGROUPS", NG))
    has_bias = bool(np.any(np.asarray(b_h)))
    key = ("nc", ngroups, has_bias)
    if key not in _cache:
        _cache[key] = _build_nc(ngroups, has_bias)
    nc = _cache[key]

    in_maps = _prep_inputs(
        pdg, feat, adj, mask, emb_table, W_in, b_in, W_h, b_h, W_out, b_out
    )
    trace = bool(int(os.environ.get("KERNEL_TRACE", "0")))
    if trace:
        try:
            tmpdir = os.environ.get("KERNEL_TRACE_DIR") or None
            res = run_bass_kernel_spmd(
                nc, in_maps, core_ids=list(range(NCORES)), trace=True, tmpdir=tmpdir
            )
            _cache["last_exec_time_ns"] = res.exec_time_ns
            _cache["last_results"] = res
        except Exception as e:
            print(f"trace run failed ({type(e).__name__}: {e}); rerunning untraced")
            _cache["last_exec_time_ns"] = None
            res = run_bass_kernel_spmd(nc, in_maps, core_ids=list(range(NCORES)))
    else:
        res = run_bass_kernel_spmd(nc, in_maps, core_ids=list(range(NCORES)))
    out = np.concatenate([res.results[c]["out"].reshape(BC) for c in range(NCORES)])
    return out.reshape(B, 1).astype(np.float32)


# revision 7
# speedup vs baseline: 1.6253x; 1.1024x over previous
"""Trainium2 Bass kernel for nn_CombinedModel_wGCN (GNN message passing).

Reference computation per event b (B=4096 events, N=128 particles):
  x = concat(feat, emb_table[pdg])          [128, 16]
  x = x @ W_in + b_in                       [128, 128]
  6x: x = relu(x @ W_h[l] + b_h[l]); x = adj @ x
  out[b] = (mask-weighted mean_i x) @ W_out + b_out

Strategy (pure data-parallel over 8 cores, 512 events each, groups of 8):
  - State kept transposed per event: Xh_e = x_e^T [d, i] (fp16). The dense
    layer is per-event matmul(lhsT=Xh_e, rhs=W_h[l]) producing [i, d'] -
    which feeds the aggregation matmul(lhsT=R_e, rhs=adjT_e) directly, so
    the whole layer chain needs NO transposes.
  - Precision: everything fp16. W_h[l] is pre-scaled by S=2^-5 (exact power
    of two - mantissa and hence quantization error untouched) to keep
    activations O(1); relu is positively homogeneous and b_h scales along,
    and the cumulative S^6 is divided back out of W_out on the host. fp16
    weights carry 11 mantissa bits, enough that a SINGLE dense matmul
    replaces a bf16 hi+lo pair (emulated end-to-end error ~4.5e-3 vs the
    f32 reference).
  - The input linear layer has no relu before the first hidden dense, so
    W_in folds into layer 0 on the host: Wc = [W_in; b_in] @ W_h[0] * S
    (f64 product, then fp16). Layer 0 consumes the DMA'd x0^T directly -
    the input-layer matmuls AND their PSUM evacuation passes disappear.
    x0^T is zero-padded from 17 to 128 rows: K=17 matmuls measure 107ns
    per (LDWEIGHTS, MATMUL) pair on HW (the pair loses its overlap) vs
    56ns at K=128, so full-K padding is a net win over the extra DMA.
  - Bias b_h: reference uses zeros; if nonzero at runtime, a rank-1
    matmul (ones (x) b*S^(l+1)) accumulates bias into the dense PSUM.
  - Masked-mean pooling folds into v = adj^T (mask/denom) (host), so the
    last aggregation is an N=1 matmul per event whose columns land in the
    group's own rotating PSUM tile and get evacuated [128,8] -> SBUF f32;
    no persistent PSUM bank, so all 8 banks serve the rotation pool.
  - PSUM->SBUF evacuations (relu of the dense output, cast-copy of the
    aggregation output) bound the kernel together with the PE. Only the
    Activation and DVE engines can read PSUM (the BIR verifier rejects
    GpSimd ops on PSUM), so passes alternate between them weighted by
    their measured pass costs (1114ns vs 1222ns per [128,1024] pass), and
    groups are sized G=8 ([128,1024] passes spanning two PSUM banks) to
    amortize each engine's fixed PSUM/SBUF access latency.
  - Groups are emitted in a 4-wide wavefront over a 4-buffer rotating PSUM
    pool (2 banks per tile = all 8 banks); the pool hold time (~1.9us:
    fill + semaphores + evacuation) over 2 tiles per group-layer caps
    throughput at 4/(2*1.9us), just above the evacuation-engine pace.
"""

import os
import numpy as np
import ml_dtypes

B, N = 4096, 128
NUM_FEAT, EMBED = 8, 8
UNITS = 128
HIDDEN = 6
VOCAB = 42
NCORES = 8
BC = B // NCORES  # events per core
G = 8  # events per group (one [128, 1024] f32 PSUM tile = 2 banks)
NG = BC // G
D0 = NUM_FEAT + EMBED + 1  # input features augmented with ones row (b_in)
WF = 4  # wavefront width (groups in flight)
SCALE = 2.0 ** -5  # per-layer weight scale keeping fp16 activations O(1)

_cache = {}


def _build_nc(ngroups, has_bias):
    import concourse.tile as tile
    from concourse import mybir, bacc

    f32 = mybir.dt.float32
    f16 = mybir.dt.float16
    bf16 = mybir.dt.bfloat16
    Relu = mybir.ActivationFunctionType.Relu
    GW = G * 128

    nc = bacc.Bacc(
        trn_type="TRN2", target_bir_lowering=False, debug=False, num_devices=NCORES
    )
    d_adjt = nc.declare_dram_parameter("adjt", [NG, 128, GW], f16, isOutput=False)
    d_x0t = nc.declare_dram_parameter("x0t", [NG, 128, GW], f16, isOutput=False)
    d_vt = nc.declare_dram_parameter("vt", [128, BC], f16, isOutput=False)
    d_wh = nc.declare_dram_parameter("wh", [HIDDEN, 128, 128], f16, isOutput=False)
    d_bh = nc.declare_dram_parameter("bh", [HIDDEN, 128], f32, isOutput=False)
    d_wout = nc.declare_dram_parameter("wout", [2, 128, 1], bf16, isOutput=False)
    d_bout = nc.declare_dram_parameter("bout", [1, 1], f32, isOutput=False)
    d_out = nc.declare_dram_parameter("out", [1, BC], f32, isOutput=True)

    with tile.TileContext(nc) as tc:
        with (
            tc.tile_pool(name="const", bufs=1) as constp,
            tc.tile_pool(name="adj", bufs=WF + 2) as adjp,
            tc.tile_pool(name="x0", bufs=WF + 2) as x0p,
            tc.tile_pool(name="work", bufs=2 * WF + 2) as workp,
            tc.tile_pool(name="ps", bufs=WF, space="PSUM") as psp,
        ):
            # ---- constants ----
            wh = []
            for l in range(HIDDEN):
                t = constp.tile([128, 128], f16, tag=f"wh{l}")
                nc.sync.dma_start(t[:], d_wh[l])
                wh.append(t)
            wouth = constp.tile([128, 1], bf16, tag="wouth")
            nc.sync.dma_start(wouth[:], d_wout[0])
            woutl = constp.tile([128, 1], bf16, tag="woutl")
            nc.sync.dma_start(woutl[:], d_wout[1])
            boutt = constp.tile([1, 1], f32, tag="bout")
            nc.sync.dma_start(boutt[:], d_bout[:])
            vsb = constp.tile([128, BC], f16, tag="vsb")
            nc.sync.dma_start(vsb[:], d_vt[:])
            pooled_sb = constp.tile([128, BC], f32, tag="pooled_sb")
            brow = []
            if has_bias:
                ones_row = constp.tile([1, 128], f16, tag="ones_row")
                nc.vector.memset(ones_row[:], 1.0)
                for l in range(HIDDEN):
                    bst = constp.tile([1, 128], f32, tag=f"bst{l}")
                    nc.sync.dma_start(bst[:], d_bh[l].rearrange("(o d) -> o d", o=1))
                    bb = constp.tile([1, GW], f16, tag=f"brow{l}")
                    for e in range(G):
                        nc.vector.tensor_copy(bb[:, e * 128 : (e + 1) * 128], bst[:])
                    brow.append(bb)

            # Evacuations alternate between the only two PSUM-capable
            # engines, weighted by measured pass cost (ACT 1114ns, DVE
            # 1222ns per [128,1024] pass) via a Bresenham accumulator.
            ebal = [0]

            def evac(dst, src, relu):
                if ebal[0] >= 0:
                    eng, ebal[0] = 0, ebal[0] - 1222
                else:
                    eng, ebal[0] = 1, ebal[0] + 1114
                if relu:
                    if eng == 0:
                        nc.scalar.activation(dst, src, Relu)
                    else:
                        nc.vector.tensor_scalar_max(dst, src, 0.0)
                else:
                    if eng == 0:
                        nc.scalar.copy(dst, src)
                    else:
                        nc.vector.tensor_copy(dst, src)

            # ---- wavefront over groups of G events ----
            xh = {}
            adjt = {}
            for gb in range(0, ngroups, WF):
                gs = range(gb, min(gb + WF, ngroups))
                for g in gs:
                    at = adjp.tile([128, GW], f16, tag="adjt")
                    nc.sync.dma_start(at[:], d_adjt[g])
                    adjt[g] = at
                    x0t = x0p.tile([128, GW], f16, tag="x0t")
                    nc.sync.dma_start(x0t[:], d_x0t[g])
                    xh[g] = x0t  # layer 0 consumes x0^T directly (fused W_in)

                for l in range(HIDDEN):
                    pd = {}
                    for g in gs:
                        p = psp.tile([128, GW], f32, tag="ps")
                        for e in range(G):
                            s = slice(e * 128, (e + 1) * 128)
                            nc.tensor.matmul(
                                p[:, s], xh[g][:, s], wh[l][:],
                                start=True, stop=not has_bias,
                            )
                        if has_bias:
                            nc.tensor.matmul(
                                p[:], ones_row[:], brow[l][:], start=False, stop=True,
                                skip_group_check=True,
                            )
                        pd[g] = p
                    rr = {}
                    for g in gs:
                        r = workp.tile([128, GW], f16, tag="r")
                        evac(r[:], pd[g][:], relu=True)
                        rr[g] = r
                    if l < HIDDEN - 1:
                        pa = {}
                        for g in gs:
                            p = psp.tile([128, GW], f32, tag="ps")
                            for e in range(G):
                                s = slice(e * 128, (e + 1) * 128)
                                nc.tensor.matmul(
                                    p[:, s], rr[g][:, s], adjt[g][:, s],
                                    start=True, stop=True,
                                )
                            pa[g] = p
                        for g in gs:
                            t = workp.tile([128, GW], f16, tag="xh")
                            evac(t[:], pa[g][:], relu=False)
                            xh[g] = t
                    else:
                        # pooling columns land in the group's own rotating
                        # tile; a tiny [128, G] pass moves them to SBUF f32
                        for g in gs:
                            p = psp.tile([128, GW], f32, tag="ps")
                            for e in range(G):
                                s = slice(e * 128, (e + 1) * 128)
                                nc.tensor.matmul(
                                    p[:, e : e + 1],
                                    rr[g][:, s],
                                    vsb[:, g * G + e : g * G + e + 1],
                                    start=True, stop=True,
                                )
                            evac(
                                pooled_sb[:, g * G : (g + 1) * G], p[:, :G],
                                relu=False,
                            )
                xh.clear()
                adjt.clear()

            # ---- final projection: out = pooled^T @ W_out + b_out ----
            phi = constp.tile([128, BC], bf16, tag="phi")
            nc.scalar.copy(phi[:], pooled_sb[:])
            plo = constp.tile([128, BC], bf16, tag="plo")
            nc.vector.tensor_tensor(
                plo[:], pooled_sb[:], phi[:], mybir.AluOpType.subtract
            )
            poutt = psp.tile([128, GW], f32, tag="ps")
            pout = poutt[:1, :BC]
            nc.tensor.matmul(pout, wouth[:], phi[:], start=True, stop=False)
            nc.tensor.matmul(pout, wouth[:], plo[:], start=False, stop=False)
            nc.tensor.matmul(pout, woutl[:], phi[:], start=False, stop=True)
            outsb = constp.tile([1, BC], f32, tag="outsb")
            nc.vector.tensor_scalar_add(outsb[:], pout[:], boutt[:])
            nc.sync.dma_start(d_out[:], outsb[:])

    nc.finalize()
    return nc


def _split2(w, dt):
    hi = w.astype(dt)
    lo = (w - hi.astype(np.float32)).astype(dt)
    return hi, lo


def _prep_inputs(pdg, feat, adj, mask, emb_table, W_in, b_in, W_h, b_h, W_out, b_out):
    bf = ml_dtypes.bfloat16
    f16 = np.float16
    pdg = np.asarray(pdg)
    feat = np.asarray(feat, dtype=np.float32)
    adj = np.asarray(adj, dtype=np.float32)
    mask = np.asarray(mask, dtype=np.float32)
    emb_table = np.asarray(emb_table, dtype=np.float32)
    W_h = np.asarray(W_h, np.float32)

    emb = emb_table[pdg]  # [B, N, EMBED]
    ones = np.ones((B, N, 1), dtype=np.float32)
    x0 = np.concatenate(
        [feat, emb, ones, np.zeros((B, N, 128 - D0), np.float32)], axis=-1
    )  # [B, N, 128] zero-padded so layer 0 is a full-K matmul
    x0t = x0.transpose(0, 2, 1)  # [B, 128, N]
    x0t4 = (
        np.ascontiguousarray(x0t.reshape(B // G, G, 128, N).transpose(0, 2, 1, 3))
        .reshape(B // G, 128, G * N)
        .astype(f16)
    )

    adjt = adj.transpose(0, 2, 1).astype(f16)  # [B, j, i]
    adjt4 = np.ascontiguousarray(
        adjt.reshape(B // G, G, N, N).transpose(0, 2, 1, 3)
    ).reshape(B // G, N, G * N)

    denom = np.clip(mask.sum(axis=1, keepdims=True), 1.0, None)
    m_scaled = (mask / denom).astype(np.float32)  # [B, N]
    v = np.matmul(m_scaled[:, None, :], adj).squeeze(1)  # [B, N]
    vt = v.T.astype(f16)  # [N, B]

    # Fuse the input linear into layer 0: Wc = [W_in; b_in] @ W_h[0] * S
    win_aug = np.concatenate(
        [np.asarray(W_in, np.float64), np.asarray(b_in, np.float64)[None, :]], axis=0
    )  # [17, 128]
    Wc = (win_aug @ W_h[0].astype(np.float64) * SCALE).astype(np.float32)
    whs = np.zeros((HIDDEN, 128, 128), np.float32)
    whs[0, :D0, :] = Wc
    whs[1:] = W_h[1:] * SCALE
    whs = whs.astype(f16)
    # scale bias rows to match the cumulative activation scale S^(l+1)
    bhs = np.asarray(b_h, np.float32) * (
        SCALE ** np.arange(1, HIDDEN + 1, dtype=np.float32)[:, None]
    )
    wout_unscaled = np.asarray(W_out, np.float32).reshape(128, 1) / (SCALE ** HIDDEN)
    wouth, woutl = _split2(wout_unscaled, bf)
    wout2 = np.stack([wouth, woutl])  # [2, 128, 1] bf16

    in_maps = []
    for c in range(NCORES):
        ev = slice(c * BC, (c + 1) * BC)
        gv = slice(c * (BC // G), (c + 1) * (BC // G))
        in_maps.append(
            {
                "adjt": adjt4[gv],
                "x0t": x0t4[gv],
                "vt": np.ascontiguousarray(vt[:, ev]),
                "wh": whs,
                "bh": bhs,
                "wout": wout2,
                "bout": np.asarray(b_out, np.float32).reshape(1, 1),
            }
        )
    return in_maps


def kernel(pdg, feat, adj, mask, emb_table, W_in, b_in, W_h, b_h, W_out, b_out):
    from concourse.bass_utils import run_bass_kernel_spmd

    ngroups = int(os.environ.get("KERNEL_NGROUPS", NG))
    has_bias = bool(np.any(np.asarray(b_h)))
    key = ("nc", ngroups, has_bias)
    if key not in _cache:
        _cache[key] = _build_nc(ngroups, has_bias)
    nc = _cache[key]

    in_maps = _prep_inputs(
        pdg, feat, adj, mask, emb_table, W_in, b_in, W_h, b_h, W_out, b_out
    )
    trace = bool(int(os.environ.get("KERNEL_TRACE", "0")))
    if trace:
        try:
            tmpdir = os.environ.get("KERNEL_TRACE_DIR") or None
            res = run_bass_kernel_spmd(
                nc, in_maps, core_ids=list(range(NCORES)), trace=True, tmpdir=tmpdir
            )
            _cache["last_exec_time_ns"] = res.exec_time_ns
            _cache["last_results"] = res
        except Exception as e:
            print(f"trace run failed ({type(e).__name__}: {e}); rerunning untraced")
            _cache["last_exec_time_ns"] = None
            res = run_bass_kernel_spmd(nc, in_maps, core_ids=list(range(NCORES)))
    else:
        res = run_bass_kernel_spmd(nc, in_maps, core_ids=list(range(NCORES)))
    out = np.concatenate([res.results[c]["out"].reshape(BC) for c in range(NCORES)])
    return out.reshape(B, 1).astype(np.float32)


# revision 8
# speedup vs baseline: 1.7141x; 1.0547x over previous
"""Trainium2 Bass kernel for nn_CombinedModel_wGCN (GNN message passing).

Reference computation per event b (B=4096 events, N=128 particles):
  x = concat(feat, emb_table[pdg])          [128, 16]
  x = x @ W_in + b_in                       [128, 128]
  6x: x = relu(x @ W_h[l] + b_h[l]); x = adj @ x
  out[b] = (mask-weighted mean_i x) @ W_out + b_out

Strategy (pure data-parallel over 8 cores, 512 events each, groups of 8):
  - State kept transposed per event: Xh_e = x_e^T [d, i] (fp16). The dense
    layer is per-event matmul(lhsT=Xh_e, rhs=W_h[l]) producing [i, d'] -
    which feeds the aggregation matmul(lhsT=R_e, rhs=adjT_e) directly, so
    the whole layer chain needs NO transposes.
  - Precision: everything fp16. W_h[l] is pre-scaled by S=2^-5 (exact power
    of two - mantissa and hence quantization error untouched) to keep
    activations O(1); relu is positively homogeneous and b_h scales along,
    and the cumulative S^6 is divided back out of W_out on the host. fp16
    weights carry 11 mantissa bits, enough that a SINGLE dense matmul
    replaces a bf16 hi+lo pair (emulated end-to-end error ~4.5e-3 vs the
    f32 reference).
  - The input linear layer has no relu before the first hidden dense, so
    W_in folds into layer 0 on the host: Wc = [W_in; b_in] @ W_h[0] * S
    (f64 product, then fp16). Layer 0 consumes the DMA'd x0^T directly -
    the input-layer matmuls AND their PSUM evacuation passes disappear.
    x0^T is zero-padded from 17 to 128 rows: K=17 matmuls measure 107ns
    per (LDWEIGHTS, MATMUL) pair on HW (the pair loses its overlap) vs
    56ns at K=128, so full-K padding is a net win over the extra DMA.
  - Bias b_h: reference uses zeros; if nonzero at runtime, a rank-1
    matmul (ones (x) b*S^(l+1)) accumulates bias into the dense PSUM.
  - Masked-mean pooling folds into v = adj^T (mask/denom) (host), so the
    last aggregation is an N=1 matmul per event whose columns land in the
    group's own rotating PSUM tile and get evacuated [128,8] -> SBUF f32;
    no persistent PSUM bank, so all 8 banks serve the rotation pool.
  - PSUM->SBUF evacuations (relu of the dense output, cast-copy of the
    aggregation output) bound the kernel together with the PE. Only the
    Activation and DVE engines can read PSUM (the BIR verifier rejects
    GpSimd ops on PSUM), so passes alternate between them weighted by
    their measured pass costs (1114ns vs 1222ns per [128,1024] pass), and
    groups are sized G=8 ([128,1024] passes spanning two PSUM banks) to
    amortize each engine's fixed PSUM/SBUF access latency.
  - Groups are emitted in a 4-wide wavefront over a 4-buffer rotating PSUM
    pool (2 banks per tile = all 8 banks); the pool hold time (~1.9us:
    fill + semaphores + evacuation) over 2 tiles per group-layer caps
    throughput at 4/(2*1.9us), just above the evacuation-engine pace.
"""

import os
import numpy as np
import ml_dtypes

B, N = 4096, 128
NUM_FEAT, EMBED = 8, 8
UNITS = 128
HIDDEN = 6
VOCAB = 42
NCORES = 8
BC = B // NCORES  # events per core
G = 8  # events per group (one [128, 1024] f32 PSUM tile = 2 banks)
NG = BC // G
D0 = NUM_FEAT + EMBED + 1  # input features augmented with ones row (b_in)
WF = 4  # wavefront width (groups in flight)
SCALE = 2.0 ** -5  # per-layer weight scale keeping fp16 activations O(1)

_cache = {}


def _build_nc(ngroups, has_bias):
    import concourse.tile as tile
    from concourse import mybir, bacc

    f32 = mybir.dt.float32
    f16 = mybir.dt.float16
    bf16 = mybir.dt.bfloat16
    Relu = mybir.ActivationFunctionType.Relu
    GW = G * 128

    nc = bacc.Bacc(
        trn_type="TRN2", target_bir_lowering=False, debug=False, num_devices=NCORES
    )
    d_adjt = nc.declare_dram_parameter("adjt", [NG, 128, GW], f16, isOutput=False)
    d_x0t = nc.declare_dram_parameter("x0t", [NG, 128, GW], f16, isOutput=False)
    d_vt = nc.declare_dram_parameter("vt", [128, BC], f16, isOutput=False)
    d_wh = nc.declare_dram_parameter("wh", [HIDDEN, 128, 128], f16, isOutput=False)
    d_bh = nc.declare_dram_parameter("bh", [HIDDEN, 128], f32, isOutput=False)
    d_wout = nc.declare_dram_parameter("wout", [2, 128, 1], bf16, isOutput=False)
    d_bout = nc.declare_dram_parameter("bout", [1, 1], f32, isOutput=False)
    d_out = nc.declare_dram_parameter("out", [1, BC], f32, isOutput=True)

    with tile.TileContext(nc) as tc:
        with (
            tc.tile_pool(name="const", bufs=1) as constp,
            tc.tile_pool(name="adj", bufs=WF + 2) as adjp,
            tc.tile_pool(name="x0", bufs=WF + 2) as x0p,
            tc.tile_pool(name="work", bufs=2 * WF + 2) as workp,
            tc.tile_pool(name="ps", bufs=WF, space="PSUM") as psp,
        ):
            # ---- constants ----
            wh = []
            for l in range(HIDDEN):
                t = constp.tile([128, 128], f16, tag=f"wh{l}")
                nc.sync.dma_start(t[:], d_wh[l])
                wh.append(t)
            wouth = constp.tile([128, 1], bf16, tag="wouth")
            nc.sync.dma_start(wouth[:], d_wout[0])
            woutl = constp.tile([128, 1], bf16, tag="woutl")
            nc.sync.dma_start(woutl[:], d_wout[1])
            boutt = constp.tile([1, 1], f32, tag="bout")
            nc.sync.dma_start(boutt[:], d_bout[:])
            vsb = constp.tile([128, BC], f16, tag="vsb")
            nc.sync.dma_start(vsb[:], d_vt[:])
            pooled_sb = constp.tile([128, BC], f32, tag="pooled_sb")
            brow = []
            if has_bias:
                ones_row = constp.tile([1, 128], f16, tag="ones_row")
                nc.vector.memset(ones_row[:], 1.0)
                for l in range(HIDDEN):
                    bst = constp.tile([1, 128], f32, tag=f"bst{l}")
                    nc.sync.dma_start(bst[:], d_bh[l].rearrange("(o d) -> o d", o=1))
                    bb = constp.tile([1, GW], f16, tag=f"brow{l}")
                    for e in range(G):
                        nc.vector.tensor_copy(bb[:, e * 128 : (e + 1) * 128], bst[:])
                    brow.append(bb)

            # Evacuations go to whichever of the two PSUM-capable engines
            # has the least accumulated work, using measured per-pass costs
            # (ns): ACT ~260 fixed + 0.833/col, DVE ~145 fixed + 1.042/col.
            eload = [0.0, 0.0]

            def evac(dst, src, relu):
                cols = dst.free_size()
                costs = (260 + 0.833 * cols, 145 + 1.042 * cols)
                eng = 0 if eload[0] + costs[0] <= eload[1] + costs[1] else 1
                eload[eng] += costs[eng]
                if relu:
                    if eng == 0:
                        nc.scalar.activation(dst, src, Relu)
                    else:
                        nc.vector.tensor_scalar_max(dst, src, 0.0)
                else:
                    if eng == 0:
                        nc.scalar.copy(dst, src)
                    else:
                        nc.vector.tensor_copy(dst, src)

            # ---- wavefront over groups of G events ----
            xh = {}
            adjt = {}
            for gb in range(0, ngroups, WF):
                gs = range(gb, min(gb + WF, ngroups))
                for g in gs:
                    at = adjp.tile([128, GW], f16, tag="adjt")
                    nc.sync.dma_start(at[:], d_adjt[g])
                    adjt[g] = at
                    x0t = x0p.tile([128, GW], f16, tag="x0t")
                    nc.sync.dma_start(x0t[:], d_x0t[g])
                    xh[g] = x0t  # layer 0 consumes x0^T directly (fused W_in)

                for l in range(HIDDEN):
                    pd = {}
                    for g in gs:
                        p = psp.tile([128, GW], f32, tag="ps")
                        for e in range(G):
                            s = slice(e * 128, (e + 1) * 128)
                            nc.tensor.matmul(
                                p[:, s], xh[g][:, s], wh[l][:],
                                start=True, stop=not has_bias,
                            )
                        if has_bias:
                            nc.tensor.matmul(
                                p[:], ones_row[:], brow[l][:], start=False, stop=True,
                                skip_group_check=True,
                            )
                        pd[g] = p
                    rr = {}
                    for g in gs:
                        r = workp.tile([128, GW], f16, tag="r")
                        evac(r[:], pd[g][:], relu=True)
                        rr[g] = r
                    if l < HIDDEN - 1:
                        pa = {}
                        for g in gs:
                            p = psp.tile([128, GW], f32, tag="ps")
                            for e in range(G):
                                s = slice(e * 128, (e + 1) * 128)
                                nc.tensor.matmul(
                                    p[:, s], rr[g][:, s], adjt[g][:, s],
                                    start=True, stop=True,
                                )
                            pa[g] = p
                        for g in gs:
                            t = workp.tile([128, GW], f16, tag="xh")
                            evac(t[:], pa[g][:], relu=False)
                            xh[g] = t
                    else:
                        # pooling columns land in the group's own rotating
                        # tile; a tiny [128, G] pass moves them to SBUF f32
                        for g in gs:
                            p = psp.tile([128, GW], f32, tag="ps")
                            for e in range(G):
                                s = slice(e * 128, (e + 1) * 128)
                                nc.tensor.matmul(
                                    p[:, e : e + 1],
                                    rr[g][:, s],
                                    vsb[:, g * G + e : g * G + e + 1],
                                    start=True, stop=True,
                                )
                            evac(
                                pooled_sb[:, g * G : (g + 1) * G], p[:, :G],
                                relu=False,
                            )
                xh.clear()
                adjt.clear()

            # ---- final projection: out = pooled^T @ W_out + b_out ----
            phi = constp.tile([128, BC], bf16, tag="phi")
            nc.scalar.copy(phi[:], pooled_sb[:])
            plo = constp.tile([128, BC], bf16, tag="plo")
            nc.vector.tensor_tensor(
                plo[:], pooled_sb[:], phi[:], mybir.AluOpType.subtract
            )
            poutt = psp.tile([128, GW], f32, tag="ps")
            pout = poutt[:1, :BC]
            nc.tensor.matmul(pout, wouth[:], phi[:], start=True, stop=False)
            nc.tensor.matmul(pout, wouth[:], plo[:], start=False, stop=False)
            nc.tensor.matmul(pout, woutl[:], phi[:], start=False, stop=True)
            outsb = constp.tile([1, BC], f32, tag="outsb")
            nc.vector.tensor_scalar_add(outsb[:], pout[:], boutt[:])
            nc.sync.dma_start(d_out[:], outsb[:])

    nc.finalize()
    return nc


def _split2(w, dt):
    hi = w.astype(dt)
    lo = (w - hi.astype(np.float32)).astype(dt)
    return hi, lo


def _prep_inputs(pdg, feat, adj, mask, emb_table, W_in, b_in, W_h, b_h, W_out, b_out):
    bf = ml_dtypes.bfloat16
    f16 = np.float16
    pdg = np.asarray(pdg)
    feat = np.asarray(feat, dtype=np.float32)
    adj = np.asarray(adj, dtype=np.float32)
    mask = np.asarray(mask, dtype=np.float32)
    emb_table = np.asarray(emb_table, dtype=np.float32)
    W_h = np.asarray(W_h, np.float32)

    emb = emb_table[pdg]  # [B, N, EMBED]
    ones = np.ones((B, N, 1), dtype=np.float32)
    x0 = np.concatenate(
        [feat, emb, ones, np.zeros((B, N, 128 - D0), np.float32)], axis=-1
    )  # [B, N, 128] zero-padded so layer 0 is a full-K matmul
    x0t = x0.transpose(0, 2, 1)  # [B, 128, N]
    x0t4 = (
        np.ascontiguousarray(x0t.reshape(B // G, G, 128, N).transpose(0, 2, 1, 3))
        .reshape(B // G, 128, G * N)
        .astype(f16)
    )

    adjt = adj.transpose(0, 2, 1).astype(f16)  # [B, j, i]
    adjt4 = np.ascontiguousarray(
        adjt.reshape(B // G, G, N, N).transpose(0, 2, 1, 3)
    ).reshape(B // G, N, G * N)

    denom = np.clip(mask.sum(axis=1, keepdims=True), 1.0, None)
    m_scaled = (mask / denom).astype(np.float32)  # [B, N]
    v = np.matmul(m_scaled[:, None, :], adj).squeeze(1)  # [B, N]
    vt = v.T.astype(f16)  # [N, B]

    # Fuse the input linear into layer 0: Wc = [W_in; b_in] @ W_h[0] * S
    win_aug = np.concatenate(
        [np.asarray(W_in, np.float64), np.asarray(b_in, np.float64)[None, :]], axis=0
    )  # [17, 128]
    Wc = (win_aug @ W_h[0].astype(np.float64) * SCALE).astype(np.float32)
    whs = np.zeros((HIDDEN, 128, 128), np.float32)
    whs[0, :D0, :] = Wc
    whs[1:] = W_h[1:] * SCALE
    whs = whs.astype(f16)
    # scale bias rows to match the cumulative activation scale S^(l+1)
    bhs = np.asarray(b_h, np.float32) * (
        SCALE ** np.arange(1, HIDDEN + 1, dtype=np.float32)[:, None]
    )
    wout_unscaled = np.asarray(W_out, np.float32).reshape(128, 1) / (SCALE ** HIDDEN)
    wouth, woutl = _split2(wout_unscaled, bf)
    wout2 = np.stack([wouth, woutl])  # [2, 128, 1] bf16

    in_maps = []
    for c in range(NCORES):
        ev = slice(c * BC, (c + 1) * BC)
        gv = slice(c * (BC // G), (c + 1) * (BC // G))
        in_maps.append(
            {
                "adjt": adjt4[gv],
                "x0t": x0t4[gv],
                "vt": np.ascontiguousarray(vt[:, ev]),
                "wh": whs,
                "bh": bhs,
                "wout": wout2,
                "bout": np.asarray(b_out, np.float32).reshape(1, 1),
            }
        )
    return in_maps


def kernel(pdg, feat, adj, mask, emb_table, W_in, b_in, W_h, b_h, W_out, b_out):
    from concourse.bass_utils import run_bass_kernel_spmd

    ngroups = int(os.environ.get("KERNEL_NGROUPS", NG))
    has_bias = bool(np.any(np.asarray(b_h)))
    key = ("nc", ngroups, has_bias)
    if key not in _cache:
        _cache[key] = _build_nc(ngroups, has_bias)
    nc = _cache[key]

    in_maps = _prep_inputs(
        pdg, feat, adj, mask, emb_table, W_in, b_in, W_h, b_h, W_out, b_out
    )
    trace = bool(int(os.environ.get("KERNEL_TRACE", "0")))
    if trace:
        try:
            tmpdir = os.environ.get("KERNEL_TRACE_DIR") or None
            res = run_bass_kernel_spmd(
                nc, in_maps, core_ids=list(range(NCORES)), trace=True, tmpdir=tmpdir
            )
            _cache["last_exec_time_ns"] = res.exec_time_ns
            _cache["last_results"] = res
        except Exception as e:
            print(f"trace run failed ({type(e).__name__}: {e}); rerunning untraced")
            _cache["last_exec_time_ns"] = None
            res = run_bass_kernel_spmd(nc, in_maps, core_ids=list(range(NCORES)))
    else:
        res = run_bass_kernel_spmd(nc, in_maps, core_ids=list(range(NCORES)))
    out = np.concatenate([res.results[c]["out"].reshape(BC) for c in range(NCORES)])
    return out.reshape(B, 1).astype(np.float32)
